# revision 27
# baseline (speedup 1.0000x reference)
"""Multi-head attention (B=2, S=2048, D=1024, H=16, hd=64) on 8 TRN2 cores.

Sharding: tensor-parallel over heads — 2 heads (a 128-wide slice of D) per
core. Each core computes Q^T/K^T/V^T projections for its head block over the
full sequence, per-head attention, and a partial output projection; the host
sums the 8 partial outputs and adds the output bias.

Layout notes (all per core):
  - x is transposed on the host; the device consumes xT [D, B*S] only.
  - Projections compute QT/KT/VT [128, S] per batch directly (weights are the
    stationary operand in natural layout, xT is the moving operand).
  - Scores are computed transposed, ST[k, q] = KT_h^T @ QT_h, so exp(ST) is
    attn^T — exactly the moving operand the ctx^T matmul needs. No on-device
    attention transposes.
  - The softmax denominator rides as a ones-column appended to V in the ctx^T
    matmul stationary (M=65): row 64 of the accumulator is sum_k exp(S).
  - Normalization: reciprocal of the denom row, broadcast across 64
    partitions with a K=1 matmul, one DVE multiply.
  - Output projection runs per head (K=64) accumulating into shared PSUM;
    partial y [B*S, D] is DMA'd out in natural orientation.
"""

import numpy as np

import concourse.bass as bass
from concourse import bacc
import concourse.mybir as mybir
import concourse.tile as tile
from concourse.bass_utils import run_bass_kernel_spmd
from concourse.masks import make_identity

F32 = mybir.dt.float32
F32R = mybir.dt.float32r
BF16 = mybir.dt.bfloat16
AF = mybir.ActivationFunctionType

N_CORES = 8
B, S, D = 2, 2048, 1024
HD = 64            # head dim
DH = 128           # per-core head block (2 heads)
NKD = D // 128     # 8  d_model k-tiles
NKS = S // 128     # 16 seq k-tiles per batch
QC = 512           # q chunk
NQC = S // QC      # 4
ROWS = B * S       # 4096

# per-stage matmul dtype knobs
ST_DT = F32R    # scores matmul operands (QT/KT tiles)
CTX_DT = F32R   # ctx matmul operands (E, V')
O_DT = F32R     # output projection operands (ctxn, Wo)
MM_DT = F32R    # legacy build-cache key component


def _emit(ctx, tc, t, mm_dt):
    nc = tc.nc
    ctx.enter_context(nc.allow_low_precision(reason="f32r matmul operand staging"))

    consts = ctx.enter_context(tc.tile_pool(name="consts", bufs=1))
    sb = ctx.enter_context(tc.tile_pool(name="sb", bufs=2))
    eb = ctx.enter_context(tc.tile_pool(name="eb", bufs=3))
    ps = ctx.enter_context(tc.tile_pool(name="ps", bufs=2, space="PSUM"))

    # ---- constants -------------------------------------------------------
    wq_sb = consts.tile([128, NKD, DH], F32R)
    wk_sb = consts.tile([128, NKD, DH], F32R)
    wv_sb = consts.tile([128, NKD, DH], F32R)
    nc.sync.dma_start(out=wq_sb, in_=t["wq"].rearrange("(kt p) m -> p kt m", p=128))
    nc.sync.dma_start(out=wk_sb, in_=t["wk"].rearrange("(kt p) m -> p kt m", p=128))
    nc.sync.dma_start(out=wv_sb, in_=t["wv"].rearrange("(kt p) m -> p kt m", p=128))
    bq_sb = consts.tile([128, 1], F32)
    bk_sb = consts.tile([128, 1], F32)
    bv_sb = consts.tile([128, 1], F32)
    nc.sync.dma_start(out=bq_sb, in_=t["bq"])
    nc.sync.dma_start(out=bk_sb, in_=t["bk"])
    nc.sync.dma_start(out=bv_sb, in_=t["bv"])
    wo_sb = consts.tile([128, D], F32R)
    nc.sync.dma_start(out=wo_sb, in_=t["wo"])
    ones_f = consts.tile([128, 128], F32)
    nc.vector.memset(ones_f, 1.0)
    zeros_f = consts.tile([128, 512], F32)
    nc.vector.memset(zeros_f, 0.0)
    ones_c = consts.tile([128, 128], F32R)
    nc.scalar.copy(ones_c, ones_f)
    # selector for the denominator broadcast: out rows 0:64 pick rhs row 64
    # (head0 recip), rows 64:128 pick rhs row 32 (head1 recip)
    zr_sel = consts.tile([128, 128], F32R)
    nc.vector.tensor_copy(zr_sel, zeros_f[:, 0:128])
    nc.vector.tensor_copy(zr_sel[64:65, 0:64], ones_c[64:65, 0:64])
    nc.vector.tensor_copy(zr_sel[32:33, 64:128], ones_c[32:33, 64:128])
    ident = consts.tile([128, 128], F32)
    make_identity(nc, ident)

    y = t["y"]

    for b in range(B):
        # ---- load xT for this batch -------------------------------------
        xt = sb.tile([128, NKD, S], F32R, tag="xt", bufs=1)
        for xc in range(8):
            lo = b * S + xc * (S // 8)
            nc.sync.dma_start(
                out=xt[:, :, xc * (S // 8):(xc + 1) * (S // 8)],
                in_=t["xT"][:, lo:lo + S // 8].rearrange("(kt p) s -> p kt s", p=128),
            )

        # ---- projections ------------------------------------------------
        # QT full [128, S]; KT split per head into zero-padded K=128 tiles
        # (kt0 rows 0:64 = head0, rows 64:128 = 0; kt1 the complement) so the
        # score matmuls run as full K=128/M=128 f32r against the shared qt.
        qt_sb = sb.tile([128, S], F32R, tag="qt", bufs=1)
        kt0_sb = sb.tile([128, S], F32R, tag="kt0", bufs=1)
        kt1_sb = sb.tile([128, S], F32R, tag="kt1", bufs=1)
        vt_sb = sb.tile([128, S], F32, tag="vt", bufs=1)
        for z in range(S // 512):
            zsl = slice(z * 512, (z + 1) * 512)
            nc.vector.tensor_copy(kt0_sb[64:128, zsl], zeros_f[64:128, :])
            nc.vector.tensor_copy(kt1_sb[0:64, zsl], zeros_f[0:64, :])
        for ck in range(S // 512):
            col = ck * 512
            csl = slice(col, col + 512)
            for w_sb, b_sb, kind in ((wq_sb, bq_sb, "q"), (wk_sb, bk_sb, "k"),
                                     (wv_sb, bv_sb, "v")):
                pp = ps.tile([128, 512], F32, tag="st", bufs=3)
                for kt in range(NKD):
                    nc.tensor.matmul(
                        pp, w_sb[:, kt, :], xt[:, kt, csl],
                        start=(kt == 0), stop=(kt == NKD - 1),
                    )
                if kind == "q":
                    nc.scalar.activation(qt_sb[:, csl], pp, AF.Identity,
                                         bias=b_sb, scale=1.0)
                elif kind == "v":
                    nc.scalar.activation(vt_sb[:, csl], pp, AF.Identity,
                                         bias=b_sb, scale=1.0)
                else:
                    nc.scalar.activation(kt0_sb[0:64, csl], pp[0:64, :],
                                         AF.Identity, bias=b_sb[0:64, :], scale=1.0)
                    nc.vector.tensor_scalar_add(kt1_sb[64:128, csl],
                                                pp[64:128, :], b_sb[64:128, :])

        # ---- V' stationaries, zero-padded to 128 cols -------------------
        # v0 cols: [V_h0 (64) | ones (1) | zeros];  out rows 0:64 = ctx_h0,
        #   row 64 = denom_h0.
        # v1 cols: [zeros (32) | ones (1) | zeros | V_h1 (64)];  out row 32 =
        #   denom_h1, rows 64:128 = ctx_h1.
        v0 = sb.tile([128, NKS, 128], F32R, tag="v0", bufs=1)
        v1 = sb.tile([128, NKS, 128], F32R, tag="v1", bufs=1)
        if b == 0:
            for kt in range(NKS):
                nc.vector.tensor_copy(v0[:, kt, :], zeros_f[:, 0:128])
                nc.vector.tensor_copy(v1[:, kt, :], zeros_f[:, 0:128])
        for kt in range(NKS):
            tp = ps.tile([128, 128], F32, tag="st", bufs=3)
            nc.tensor.transpose(tp, vt_sb[:, kt * 128:(kt + 1) * 128], ident)
            nc.vector.tensor_copy(v0[:, kt, 0:64], tp[:, 0:64])
            nc.vector.tensor_copy(v1[:, kt, 64:128], tp[:, 64:128])
        nc.vector.tensor_copy(v0[:, :, 64:65], ones_c[:, 0:NKS].unsqueeze(-1))
        nc.vector.tensor_copy(v1[:, :, 32:33], ones_c[:, 0:NKS].unsqueeze(-1))

        # ---- attention ---------------------------------------------------
        cn = sb.tile([128, S], F32R, tag="cn", bufs=2)
        kts = (kt0_sb, kt1_sb)
        vvs = (v0, v1)
        pend = []

        def _finish_norm(item):
            qsl_, c0_, c1_, rrr_ = item
            bc = ps.tile([128, QC], F32, tag="st", bufs=3, name="bc")
            nc.tensor.matmul(bc, zr_sel, rrr_, start=True, stop=True)
            bcs = sb.tile([128, QC], F32, tag="bcs", bufs=2, name="bcs")
            nc.vector.tensor_copy(bcs, bc)
            nc.vector.tensor_mul(cn[0:64, qsl_], c0_[0:64, :], bcs[0:64, :])
            nc.vector.tensor_mul(cn[64:128, qsl_], c1_[64:128, :], bcs[64:128, :])
            # output projection for this chunk (4 row-tiles of 128)
            for qt in range(qsl_.start // 128, qsl_.stop // 128):
                qtl = slice(qt * 128, (qt + 1) * 128)
                ys = eb.tile([128, D], F32, tag="ys", bufs=3, name="ys")
                for ec in range(D // 512):
                    esl = slice(ec * 512, (ec + 1) * 512)
                    yp = ps.tile([128, 512], F32, tag="st", bufs=3, name="yp")
                    nc.tensor.matmul(yp, cn[:, qtl], wo_sb[:, esl],
                                     start=True, stop=True)
                    nc.vector.tensor_copy(ys[:, esl], yp)
                nc.sync.dma_start(
                    out=y[b * S + qt * 128: b * S + (qt + 1) * 128, :], in_=ys)
        for qc in range(NQC):
            qsl = slice(qc * QC, (qc + 1) * QC)
            cps = []
            for h in range(2):
                cp = ps.tile([128, QC], F32, tag="ctx", bufs=2, name=f"cp{h}")
                cps.append(cp)
            for ktp in range(NKS // 2):
                sts, ees = [], []
                for h in range(2):
                    st = ps.tile([128, 2 * QC], F32, tag="st", bufs=3,
                                 name=f"st{h}")
                    sts.append(st)
                for j in range(2):
                    kt = ktp * 2 + j
                    for h in range(2):
                        nc.tensor.matmul(
                            sts[h][:, j * QC:(j + 1) * QC],
                            kts[h][:, kt * 128:(kt + 1) * 128],
                            qt_sb[:, qsl],
                            start=True, stop=True,
                        )
                for h in range(2):
                    ee = eb.tile([128, 2 * QC], F32R, tag="e", bufs=5,
                                 name=f"ee{h}")
                    ees.append(ee)
                    nc.scalar.activation(ee, sts[h], AF.Exp)
                for h in range(2):
                    for j in range(2):
                        kt = ktp * 2 + j
                        nc.tensor.matmul(
                            cps[h], vvs[h][:, kt, :],
                            ees[h][:, j * QC:(j + 1) * QC],
                            start=(ktp == 0 and j == 0),
                            stop=(ktp == NKS // 2 - 1 and j == 1),
                        )
            # Snapshot both accumulators to SBUF (releases the PSUM slots),
            # start the reciprocal chain, but defer the broadcast matmul +
            # final multiplies by one chunk so the PE queue never stalls on
            # the (in-order) broadcast waiting for the reciprocal.
            cpc0 = sb.tile([128, QC], F32, tag="cpc0", bufs=2)
            nc.vector.tensor_copy(cpc0, cps[0])
            cpc1 = sb.tile([128, QC], F32, tag="cpc1", bufs=2)
            nc.scalar.copy(cpc1, cps[1])
            den = sb.tile([128, QC], F32, tag="den", bufs=2)
            nc.vector.tensor_copy(den[64:65, :], cpc0[64:65, :])
            nc.vector.tensor_copy(den[32:33, :], cpc1[32:33, :])
            rr = sb.tile([128, QC], F32, tag="rr", bufs=2)
            nc.vector.reciprocal_approx_fast(out=rr, in_=den)
            rr_r = sb.tile([128, QC], F32R, tag="rr_r", bufs=2)
            nc.vector.tensor_copy(rr_r[32:33, :], rr[32:33, :])
            nc.vector.tensor_copy(rr_r[64:65, :], rr[64:65, :])
            pend.append((qsl, cpc0, cpc1, rr_r))
            if len(pend) > 1:
                _finish_norm(pend.pop(0))

        while pend:
            _finish_norm(pend.pop(0))



def _build_nc(mm_dt=MM_DT):
    from contextlib import ExitStack

    nc = bacc.Bacc("TRN2", debug=False)
    t = {}
    t["xT"] = nc.dram_tensor("xT", [D, ROWS], F32R, kind="ExternalInput").ap()
    for n in ("wq", "wk", "wv"):
        t[n] = nc.dram_tensor(n, [D, DH], F32R, kind="ExternalInput").ap()
    for n in ("bq", "bk", "bv"):
        t[n] = nc.dram_tensor(n, [DH, 1], F32, kind="ExternalInput").ap()
    t["wo"] = nc.dram_tensor("wo", [DH, D], O_DT, kind="ExternalInput").ap()
    t["y"] = nc.dram_tensor("y", [ROWS, D], F32, kind="ExternalOutput").ap()

    with tile.TileContext(nc) as tc:
        with ExitStack() as ctx:
            _emit(ctx, tc, t, mm_dt)
    nc.compile()
    return nc


_NC_CACHE = {}


def _get_nc(mm_dt=MM_DT):
    key = f"{mm_dt}-{ST_DT}-{CTX_DT}-{O_DT}"
    if key not in _NC_CACHE:
        _NC_CACHE[key] = _build_nc(mm_dt)
    return _NC_CACHE[key]


def _cast_for(dt, arr):
    return arr.astype(mybir.dt.np(dt))


def _in_maps(x, Wq, bq, Wk, bk, Wv, bv, Wo, bo):
    x = np.asarray(x, dtype=np.float32)
    xT_bf = np.ascontiguousarray(x.reshape(ROWS, D).T)
    Wq, bq = np.asarray(Wq, np.float32), np.asarray(bq, np.float32)
    Wk, bk = np.asarray(Wk, np.float32), np.asarray(bk, np.float32)
    Wv, bv = np.asarray(Wv, np.float32), np.asarray(bv, np.float32)
    Wo = np.asarray(Wo, np.float32)
    maps = []
    for c in range(N_CORES):
        sl = slice(c * DH, (c + 1) * DH)
        maps.append({
            "xT": xT_bf,
            "wq": np.ascontiguousarray(Wq[:, sl]) / 8.0,
            "bq": (bq[sl] / 8.0).reshape(DH, 1).copy(),
            "wk": np.ascontiguousarray(Wk[:, sl]),
            "bk": bk[sl].reshape(DH, 1).copy(),
            "wv": np.ascontiguousarray(Wv[:, sl]),
            "bv": bv[sl].reshape(DH, 1).copy(),
            "wo": _cast_for(O_DT, np.ascontiguousarray(Wo[sl])),
        })
    return maps


def _run(trace=False, **inputs):
    bo = np.asarray(inputs["bo"], np.float32)
    maps = _in_maps(**inputs)
    nc = _get_nc()
    res = run_bass_kernel_spmd(nc, maps, core_ids=list(range(N_CORES)), trace=trace)
    y = np.zeros((ROWS, D), np.float64)
    for m in res.results:
        y += m["y"].astype(np.float64)
    y = (y + bo.astype(np.float64)).astype(np.float32).reshape(B, S, D)
    return y, res


def kernel(**inputs):
    y, _ = _run(trace=False, **inputs)
    return y


# revision 28
# speedup vs baseline: 1.0201x; 1.0201x over previous
"""Multi-head attention (B=2, S=2048, D=1024, H=16, hd=64) on 8 TRN2 cores.

Sharding: tensor-parallel over heads — 2 heads (a 128-wide slice of D) per
core. Each core computes Q^T/K^T/V^T projections for its head block over the
full sequence, per-head attention, and a partial output projection; the host
sums the 8 partial outputs and adds the output bias.

Layout notes (all per core):
  - x is transposed on the host; the device consumes xT [D, B*S] only.
  - Projections compute QT/KT/VT [128, S] per batch directly (weights are the
    stationary operand in natural layout, xT is the moving operand).
  - Scores are computed transposed, ST[k, q] = KT_h^T @ QT_h, so exp(ST) is
    attn^T — exactly the moving operand the ctx^T matmul needs. No on-device
    attention transposes.
  - The softmax denominator rides as a ones-column appended to V in the ctx^T
    matmul stationary (M=65): row 64 of the accumulator is sum_k exp(S).
  - Normalization: reciprocal of the denom row, broadcast across 64
    partitions with a K=1 matmul, one DVE multiply.
  - Output projection runs per head (K=64) accumulating into shared PSUM;
    partial y [B*S, D] is DMA'd out in natural orientation.
"""

import numpy as np

import concourse.bass as bass
from concourse import bacc
import concourse.mybir as mybir
import concourse.tile as tile
from concourse.bass_utils import run_bass_kernel_spmd
from concourse.masks import make_identity

F32 = mybir.dt.float32
F32R = mybir.dt.float32r
BF16 = mybir.dt.bfloat16
AF = mybir.ActivationFunctionType

N_CORES = 8
B, S, D = 2, 2048, 1024
HD = 64            # head dim
DH = 128           # per-core head block (2 heads)
NKD = D // 128     # 8  d_model k-tiles
NKS = S // 128     # 16 seq k-tiles per batch
QC = 512           # q chunk
NQC = S // QC      # 4
ROWS = B * S       # 4096

# per-stage matmul dtype knobs
ST_DT = F32R    # scores matmul operands (QT/KT tiles)
CTX_DT = F32R   # ctx matmul operands (E, V')
O_DT = F32R     # output projection operands (ctxn, Wo)
MM_DT = F32R    # legacy build-cache key component


def _emit(ctx, tc, t, mm_dt):
    nc = tc.nc
    ctx.enter_context(nc.allow_low_precision(reason="f32r matmul operand staging"))

    consts = ctx.enter_context(tc.tile_pool(name="consts", bufs=1))
    sb = ctx.enter_context(tc.tile_pool(name="sb", bufs=2))
    eb = ctx.enter_context(tc.tile_pool(name="eb", bufs=3))
    ps = ctx.enter_context(tc.tile_pool(name="ps", bufs=2, space="PSUM"))

    # ---- constants -------------------------------------------------------
    wq_sb = consts.tile([128, NKD, DH], F32R)
    wk_sb = consts.tile([128, NKD, DH], F32R)
    wv_sb = consts.tile([128, NKD, DH], F32R)
    nc.sync.dma_start(out=wq_sb, in_=t["wq"].rearrange("(kt p) m -> p kt m", p=128))
    nc.sync.dma_start(out=wk_sb, in_=t["wk"].rearrange("(kt p) m -> p kt m", p=128))
    nc.sync.dma_start(out=wv_sb, in_=t["wv"].rearrange("(kt p) m -> p kt m", p=128))
    bq_sb = consts.tile([128, 1], F32)
    bk_sb = consts.tile([128, 1], F32)
    bv_sb = consts.tile([128, 1], F32)
    nc.sync.dma_start(out=bq_sb, in_=t["bq"])
    nc.sync.dma_start(out=bk_sb, in_=t["bk"])
    nc.sync.dma_start(out=bv_sb, in_=t["bv"])
    wo_sb = consts.tile([128, D], F32R)
    nc.sync.dma_start(out=wo_sb, in_=t["wo"])
    ones_f = consts.tile([128, 128], F32)
    nc.vector.memset(ones_f, 1.0)
    zeros_f = consts.tile([128, 512], F32)
    nc.vector.memset(zeros_f, 0.0)
    ones_c = consts.tile([128, 128], F32R)
    nc.scalar.copy(ones_c, ones_f)
    # selector for the denominator broadcast: out rows 0:64 pick rhs row 64
    # (head0 recip), rows 64:128 pick rhs row 32 (head1 recip)
    zr_sel = consts.tile([128, 128], F32R)
    nc.vector.tensor_copy(zr_sel, zeros_f[:, 0:128])
    nc.vector.tensor_copy(zr_sel[64:65, 0:64], ones_c[64:65, 0:64])
    nc.vector.tensor_copy(zr_sel[32:33, 64:128], ones_c[32:33, 64:128])
    ident = consts.tile([128, 128], F32)
    make_identity(nc, ident)

    y = t["y"]

    for b in range(B):
        # ---- load xT for this batch -------------------------------------
        xt = sb.tile([128, NKD, S], F32R, tag="xt", bufs=1)
        for xc in range(8):
            lo = b * S + xc * (S // 8)
            nc.sync.dma_start(
                out=xt[:, :, xc * (S // 8):(xc + 1) * (S // 8)],
                in_=t["xT"][:, lo:lo + S // 8].rearrange("(kt p) s -> p kt s", p=128),
            )

        # ---- projections ------------------------------------------------
        # QT full [128, S]; KT split per head into zero-padded K=128 tiles
        # (kt0 rows 0:64 = head0, rows 64:128 = 0; kt1 the complement) so the
        # score matmuls run as full K=128/M=128 f32r against the shared qt.
        qt_sb = sb.tile([128, S], F32R, tag="qt", bufs=1)
        kt0_sb = sb.tile([128, S], F32R, tag="kt0", bufs=1)
        kt1_sb = sb.tile([128, S], F32R, tag="kt1", bufs=1)
        vt_sb = sb.tile([128, S], F32, tag="vt", bufs=1)
        for z in range(S // 512):
            zsl = slice(z * 512, (z + 1) * 512)
            nc.vector.tensor_copy(kt0_sb[64:128, zsl], zeros_f[64:128, :])
            nc.vector.tensor_copy(kt1_sb[0:64, zsl], zeros_f[0:64, :])
        for ck in range(S // 512):
            col = ck * 512
            csl = slice(col, col + 512)
            for w_sb, b_sb, kind in ((wq_sb, bq_sb, "q"), (wk_sb, bk_sb, "k"),
                                     (wv_sb, bv_sb, "v")):
                pp = ps.tile([128, 512], F32, tag="st", bufs=3)
                for kt in range(NKD):
                    nc.tensor.matmul(
                        pp, w_sb[:, kt, :], xt[:, kt, csl],
                        start=(kt == 0), stop=(kt == NKD - 1),
                    )
                if kind == "q":
                    nc.scalar.activation(qt_sb[:, csl], pp, AF.Identity,
                                         bias=b_sb, scale=1.0)
                elif kind == "v":
                    nc.scalar.activation(vt_sb[:, csl], pp, AF.Identity,
                                         bias=b_sb, scale=1.0)
                else:
                    nc.scalar.activation(kt0_sb[0:64, csl], pp[0:64, :],
                                         AF.Identity, bias=b_sb[0:64, :], scale=1.0)
                    nc.vector.tensor_scalar_add(kt1_sb[64:128, csl],
                                                pp[64:128, :], b_sb[64:128, :])

        # ---- V' stationaries, zero-padded to 128 cols -------------------
        # v0 cols: [V_h0 (64) | ones (1) | zeros];  out rows 0:64 = ctx_h0,
        #   row 64 = denom_h0.
        # v1 cols: [zeros (32) | ones (1) | zeros | V_h1 (64)];  out row 32 =
        #   denom_h1, rows 64:128 = ctx_h1.
        v0 = sb.tile([128, NKS, 128], F32R, tag="v0", bufs=1)
        v1 = sb.tile([128, NKS, 128], F32R, tag="v1", bufs=1)
        if b == 0:
            for kt in range(NKS):
                nc.vector.tensor_copy(v0[:, kt, :], zeros_f[:, 0:128])
                nc.vector.tensor_copy(v1[:, kt, :], zeros_f[:, 0:128])
        for kt in range(NKS):
            tp = ps.tile([128, 128], F32, tag="st", bufs=3)
            nc.tensor.transpose(tp, vt_sb[:, kt * 128:(kt + 1) * 128], ident)
            nc.vector.tensor_copy(v0[:, kt, 0:64], tp[:, 0:64])
            nc.vector.tensor_copy(v1[:, kt, 64:128], tp[:, 64:128])
        nc.vector.tensor_copy(v0[:, :, 64:65], ones_c[:, 0:NKS].unsqueeze(-1))
        nc.vector.tensor_copy(v1[:, :, 32:33], ones_c[:, 0:NKS].unsqueeze(-1))

        # ---- attention ---------------------------------------------------
        cn = sb.tile([128, S], F32R, tag="cn", bufs=1)
        kts = (kt0_sb, kt1_sb)
        vvs = (v0, v1)
        pend = []

        def _finish_norm(item):
            qsl_, c0_, c1_, rrr_ = item
            bc = ps.tile([128, QC], F32, tag="st", bufs=3, name="bc")
            nc.tensor.matmul(bc, zr_sel, rrr_, start=True, stop=True)
            bcs = sb.tile([128, QC], F32, tag="bcs", bufs=2, name="bcs")
            nc.vector.tensor_copy(bcs, bc)
            nc.vector.tensor_mul(cn[0:64, qsl_], c0_[0:64, :], bcs[0:64, :])
            nc.vector.tensor_mul(cn[64:128, qsl_], c1_[64:128, :], bcs[64:128, :])
        for qc in range(NQC):
            qsl = slice(qc * QC, (qc + 1) * QC)
            cps = []
            for h in range(2):
                cp = ps.tile([128, QC], F32, tag="ctx", bufs=2, name=f"cp{h}")
                cps.append(cp)
            for ktp in range(NKS // 2):
                sts, ees = [], []
                for h in range(2):
                    st = ps.tile([128, 2 * QC], F32, tag="st", bufs=3,
                                 name=f"st{h}")
                    sts.append(st)
                for j in range(2):
                    kt = ktp * 2 + j
                    for h in range(2):
                        nc.tensor.matmul(
                            sts[h][:, j * QC:(j + 1) * QC],
                            kts[h][:, kt * 128:(kt + 1) * 128],
                            qt_sb[:, qsl],
                            start=True, stop=True,
                        )
                for h in range(2):
                    ee = eb.tile([128, 2 * QC], F32R, tag="e", bufs=5,
                                 name=f"ee{h}")
                    ees.append(ee)
                    nc.scalar.activation(ee, sts[h], AF.Exp)
                for h in range(2):
                    for j in range(2):
                        kt = ktp * 2 + j
                        nc.tensor.matmul(
                            cps[h], vvs[h][:, kt, :],
                            ees[h][:, j * QC:(j + 1) * QC],
                            start=(ktp == 0 and j == 0),
                            stop=(ktp == NKS // 2 - 1 and j == 1),
                        )
            # Snapshot both accumulators to SBUF (releases the PSUM slots),
            # start the reciprocal chain, but defer the broadcast matmul +
            # final multiplies by one chunk so the PE queue never stalls on
            # the (in-order) broadcast waiting for the reciprocal.
            cpc0 = sb.tile([128, QC], F32, tag="cpc0", bufs=2)
            nc.vector.tensor_copy(cpc0, cps[0])
            cpc1 = sb.tile([128, QC], F32, tag="cpc1", bufs=2)
            nc.scalar.copy(cpc1, cps[1])
            den = sb.tile([128, QC], F32, tag="den", bufs=2)
            nc.vector.tensor_copy(den[64:65, :], cpc0[64:65, :])
            nc.vector.tensor_copy(den[32:33, :], cpc1[32:33, :])
            rr = sb.tile([128, QC], F32, tag="rr", bufs=2)
            nc.vector.reciprocal_approx_fast(out=rr, in_=den)
            rr_r = sb.tile([128, QC], F32R, tag="rr_r", bufs=2)
            nc.vector.tensor_copy(rr_r[32:33, :], rr[32:33, :])
            nc.vector.tensor_copy(rr_r[64:65, :], rr[64:65, :])
            pend.append((qsl, cpc0, cpc1, rr_r))
            if len(pend) > 1:
                _finish_norm(pend.pop(0))

        while pend:
            _finish_norm(pend.pop(0))

        # ---- output projection (fused heads, K=128) ---------------------
        for qt in range(S // 128):
            qtl = slice(qt * 128, (qt + 1) * 128)
            ys = eb.tile([128, D], F32, tag="ys", bufs=3)
            for ec in range(D // 512):
                esl = slice(ec * 512, (ec + 1) * 512)
                yp = ps.tile([128, 512], F32, tag="st", bufs=3)
                nc.tensor.matmul(yp, cn[:, qtl], wo_sb[:, esl],
                                 start=True, stop=True)
                nc.vector.tensor_copy(ys[:, esl], yp)
            nc.sync.dma_start(out=y[b * S + qt * 128: b * S + (qt + 1) * 128, :], in_=ys)



def _build_nc(mm_dt=MM_DT):
    from contextlib import ExitStack

    nc = bacc.Bacc("TRN2", debug=False)
    t = {}
    t["xT"] = nc.dram_tensor("xT", [D, ROWS], F32R, kind="ExternalInput").ap()
    for n in ("wq", "wk", "wv"):
        t[n] = nc.dram_tensor(n, [D, DH], F32R, kind="ExternalInput").ap()
    for n in ("bq", "bk", "bv"):
        t[n] = nc.dram_tensor(n, [DH, 1], F32, kind="ExternalInput").ap()
    t["wo"] = nc.dram_tensor("wo", [DH, D], O_DT, kind="ExternalInput").ap()
    t["y"] = nc.dram_tensor("y", [ROWS, D], F32, kind="ExternalOutput").ap()

    with tile.TileContext(nc) as tc:
        with ExitStack() as ctx:
            _emit(ctx, tc, t, mm_dt)
    nc.compile()
    return nc


_NC_CACHE = {}


def _get_nc(mm_dt=MM_DT):
    key = f"{mm_dt}-{ST_DT}-{CTX_DT}-{O_DT}"
    if key not in _NC_CACHE:
        _NC_CACHE[key] = _build_nc(mm_dt)
    return _NC_CACHE[key]


def _cast_for(dt, arr):
    return arr.astype(mybir.dt.np(dt))


def _in_maps(x, Wq, bq, Wk, bk, Wv, bv, Wo, bo):
    x = np.asarray(x, dtype=np.float32)
    xT_bf = np.ascontiguousarray(x.reshape(ROWS, D).T)
    Wq, bq = np.asarray(Wq, np.float32), np.asarray(bq, np.float32)
    Wk, bk = np.asarray(Wk, np.float32), np.asarray(bk, np.float32)
    Wv, bv = np.asarray(Wv, np.float32), np.asarray(bv, np.float32)
    Wo = np.asarray(Wo, np.float32)
    maps = []
    for c in range(N_CORES):
        sl = slice(c * DH, (c + 1) * DH)
        maps.append({
            "xT": xT_bf,
            "wq": np.ascontiguousarray(Wq[:, sl]) / 8.0,
            "bq": (bq[sl] / 8.0).reshape(DH, 1).copy(),
            "wk": np.ascontiguousarray(Wk[:, sl]),
            "bk": bk[sl].reshape(DH, 1).copy(),
            "wv": np.ascontiguousarray(Wv[:, sl]),
            "bv": bv[sl].reshape(DH, 1).copy(),
            "wo": _cast_for(O_DT, np.ascontiguousarray(Wo[sl])),
        })
    return maps


def _run(trace=False, **inputs):
    bo = np.asarray(inputs["bo"], np.float32)
    maps = _in_maps(**inputs)
    nc = _get_nc()
    res = run_bass_kernel_spmd(nc, maps, core_ids=list(range(N_CORES)), trace=trace)
    y = np.zeros((ROWS, D), np.float64)
    for m in res.results:
        y += m["y"].astype(np.float64)
    y = (y + bo.astype(np.float64)).astype(np.float32).reshape(B, S, D)
    return y, res


def kernel(**inputs):
    y, _ = _run(trace=False, **inputs)
    return y
